# revision 7
# baseline (speedup 1.0000x reference)
"""Entropy-regularized attention (standard MHA fwd) on 8 trn2 cores.

Sharding: core c -> batch b=c//4, head-group g=c%4 (4 of 16 heads).
Each core computes q/k/v for its 256-wide head-group slice, transposed-
layout attention (scores^T = K^T-stationary matmuls, exp on ACT, AV with
v-stationary producing avT), then a row-split Wo partial product.
Host sums the 4 partials per batch and adds bo (the "all-reduce").

All matmuls run as float32r (1 cycle/row vs 4 for plain fp32).
"""

import sys

for _p in ("/opt/trn_rl_repo", "/root/.axon_site/_ro/trn_rl_repo"):
    if _p not in sys.path:
        sys.path.insert(0, _p)

import numpy as np

import concourse.bass as bass
import concourse.mybir as mybir
import concourse.tile as tile
from concourse import bacc

P = 128
S = 2048  # sequence length
D = 1024  # hidden
DG = 256  # per-core head-group width (4 heads x 64)
HD = 64
NHL = 4  # heads per core
KT_D = D // P  # 8 contraction tiles for projections
ST = S // P  # 16 sequence tiles
QG = 1024  # qi group size (PSUM budget: scores 2x2 banks + av 2 + rb 2)
NQG = S // QG

F32 = mybir.dt.float32
F32R = mybir.dt.float32r


def build_nc():
    nc = bacc.Bacc(None, target_bir_lowering=False)

    xT = nc.dram_tensor("xT", [D, S], F32R, kind="ExternalInput")
    wq = nc.dram_tensor("wq", [D, DG], F32R, kind="ExternalInput")
    wk = nc.dram_tensor("wk", [D, DG], F32R, kind="ExternalInput")
    wv = nc.dram_tensor("wv", [D, DG], F32R, kind="ExternalInput")
    wo = nc.dram_tensor("wo", [DG, D], F32R, kind="ExternalInput")
    bq = nc.dram_tensor("bq", [P, 2], F32, kind="ExternalInput")
    bk = nc.dram_tensor("bk", [P, 2], F32, kind="ExternalInput")
    bv = nc.dram_tensor("bv", [1, DG], F32R, kind="ExternalInput")
    out = nc.dram_tensor("out", [S, D], F32, kind="ExternalOutput")

    with tile.TileContext(nc) as tc:
        _body(tc, nc, xT, wq, wk, wv, wo, bq, bk, bv, out)
    nc.compile()
    return nc


def _body(tc, nc, xT, wq, wk, wv, wo, bq, bk, bv, out):
    from contextlib import ExitStack

    with ExitStack() as ctx:
        ctx.enter_context(
            nc.allow_low_precision(
                reason="float32r tiles feed fp32r matmuls; accum stays fp32 in PSUM"
            )
        )
        persist = ctx.enter_context(tc.tile_pool(name="persist", bufs=1))

        qT_sb = persist.tile([P, 2, S], F32R)
        kT_sb = persist.tile([P, 2, S], F32R)
        v_sb = persist.tile([P, ST, NHL * 65], F32R)  # 65-striped: col 64 = ones
        avT = [
            persist.tile([P, 2, QG], F32R, tag=f"avT{g}", name=f"avT{g}")
            for g in range(NQG)
        ]
        wo_sb = persist.tile([P, 2, D], F32R)
        ones_row = persist.tile([1, P], F32R)

        nc.sync.dma_start(wo_sb[:], wo.rearrange("(kt p) n -> p kt n", p=P))
        # memset can't emit float32r; stage fp32 ones and copy-cast (rounds)
        ones_f32 = persist.tile([P, P], F32)
        nc.vector.memset(ones_f32[:], 1.0)
        nc.vector.tensor_copy(ones_row[:], ones_f32[0:1, :])
        nc.vector.tensor_copy(
            v_sb.rearrange("p st (h w) -> p st h w", w=65)[:, :, :, 64],
            ones_f32[:, 0:64].rearrange("p (st h) -> p st h", h=4),
        )

        # ---- Phase B: projections qT/kT [256, S], v [S, 256] ----
        with ExitStack() as bctx:
            wpool = bctx.enter_context(tc.tile_pool(name="wpool", bufs=1))
            xpool = bctx.enter_context(tc.tile_pool(name="xpool", bufs=1))
            psB = bctx.enter_context(
                tc.tile_pool(name="psB", bufs=2, space="PSUM")
            )

            xT_sb = xpool.tile([P, KT_D, S], F32R)
            nc.sync.dma_start(xT_sb[:], xT.rearrange("(kt p) s -> p kt s", p=P))

            wq_sb = wpool.tile([P, KT_D, DG], F32R, tag="wq")
            wk_sb = wpool.tile([P, KT_D, DG], F32R, tag="wk")
            wv_sb = wpool.tile([P, KT_D, DG], F32R, tag="wv")
            nc.sync.dma_start(wq_sb[:], wq.rearrange("(kt p) n -> p kt n", p=P))
            nc.sync.dma_start(wk_sb[:], wk.rearrange("(kt p) n -> p kt n", p=P))
            nc.sync.dma_start(wv_sb[:], wv.rearrange("(kt p) n -> p kt n", p=P))
            bq_sb = wpool.tile([P, 2], F32, tag="bq")
            bk_sb = wpool.tile([P, 2], F32, tag="bk")
            bv_sb = wpool.tile([1, DG], F32R, tag="bv")
            nc.sync.dma_start(bq_sb[:], bq[:])
            nc.sync.dma_start(bk_sb[:], bk[:])
            nc.sync.dma_start(bv_sb[:], bv[:])

            # qT/kT: out[d', s] = W[:, d'].T @ xT ; bias folded into eviction
            for wsb, bsb, dest in ((wq_sb, bq_sb, qT_sb), (wk_sb, bk_sb, kT_sb)):
                for mt in range(2):
                    for nq in range(S // 512):
                        ps = psB.tile([P, 512], F32, tag="qkps")
                        for kt in range(KT_D):
                            nc.tensor.matmul(
                                ps[:],
                                (wsb[:, kt, mt * P : (mt + 1) * P]),
                                (xT_sb[:, kt, nq * 512 : (nq + 1) * 512]),
                                start=(kt == 0),
                                stop=(kt == KT_D - 1),
                            )
                        nc.vector.tensor_scalar_add(
                            dest[:, mt, nq * 512 : (nq + 1) * 512],
                            ps[:],
                            bsb[:, mt : mt + 1],
                        )

            # v natural [s, d'] ; bias via K=1 ones matmul
            for st in range(ST):
                ps = psB.tile([P, DG], F32, tag="vps")
                for kt in range(KT_D):
                    nc.tensor.matmul(
                        ps[:],
                        (xT_sb[:, kt, st * P : (st + 1) * P]),
                        (wv_sb[:, kt, :]),
                        start=(kt == 0),
                        stop=False,
                    )
                nc.tensor.matmul(
                    ps[:],
                    (ones_row[0:1, 0:P]),
                    (bv_sb[0:1, :]),
                    start=False,
                    stop=True,
                )
                nc.vector.tensor_copy(
                    v_sb.rearrange("p st (h w) -> p st h w", w=65)[
                        :, st, :, 0:64
                    ],
                    ps.rearrange("p (h w) -> p h w", w=64),
                )

        # ---- Phase C: attention, + Phase D: projection (overlapped) ----
        with ExitStack() as cctx:
            expool = cctx.enter_context(tc.tile_pool(name="expool", bufs=3))
            npool = cctx.enter_context(tc.tile_pool(name="npool", bufs=2))
            opool = cctx.enter_context(tc.tile_pool(name="opool", bufs=3))
            ps_sc = cctx.enter_context(
                tc.tile_pool(name="ps_sc", bufs=2, space="PSUM")
            )
            ps_av = cctx.enter_context(
                tc.tile_pool(name="ps_av", bufs=1, space="PSUM")
            )
            ps_o = cctx.enter_context(
                tc.tile_pool(name="ps_o", bufs=2, space="PSUM")
            )

            for qg in range(NQG):
                q0 = qg * QG
                for h in range(NHL):
                    mt, po = h // 2, (h % 2) * 64
                    av = ps_av.tile([P, QG], F32, tag="av")
                    for kt in range(ST):
                        sc = ps_sc.tile([P, QG], F32, tag="sc")
                        for nq in range(QG // 512):
                            nc.tensor.matmul(
                                sc[:, nq * 512 : (nq + 1) * 512],
                                (kT_sb[po : po + 64, mt, kt * P : (kt + 1) * P]),
                                (
                                    qT_sb[
                                        po : po + 64,
                                        mt,
                                        q0 + nq * 512 : q0 + (nq + 1) * 512,
                                    ]
                                ),
                                start=True,
                                stop=True,
                            )
                        ex = expool.tile([P, QG], F32R, tag="ex")
                        nc.scalar.activation(
                            ex[:],
                            sc[:],
                            mybir.ActivationFunctionType.Exp,
                            scale=0.125,
                        )
                        for nq in range(QG // 512):
                            nc.tensor.matmul(
                                av[0:65, nq * 512 : (nq + 1) * 512],
                                (v_sb[:, kt, h * 65 : h * 65 + 65]),
                                (ex[:, nq * 512 : (nq + 1) * 512]),
                                start=(kt == 0),
                                stop=(kt == ST - 1),
                            )
                    # normalize: avT_norm = av[0:64] * (1 / av[64]) bcast over d
                    r_row = npool.tile([1, QG], F32R, tag="rrow")
                    nc.vector.reciprocal(r_row[:], av[64:65, :])
                    rb = ps_sc.tile([P, QG], F32, tag="sc")
                    for nq in range(QG // 512):
                        nc.tensor.matmul(
                            rb[0:64, nq * 512 : (nq + 1) * 512],
                            (ones_row[0:1, 0:64]),
                            (r_row[0:1, nq * 512 : (nq + 1) * 512]),
                            start=True,
                            stop=True,
                        )
                    u_sb = npool.tile([64, QG], F32R, tag="usb")
                    nc.vector.tensor_copy(u_sb[:], av[0:64, :])
                    nc.vector.tensor_mul(
                        out=avT[qg][po : po + 64, mt, :],
                        in0=u_sb[:],
                        in1=rb[0:64, :],
                    )

                # Phase D for this qi-group (st tiles qg*8 .. qg*8+8)
                for sti in range(QG // P):
                    st = qg * (QG // P) + sti
                    po_ps = ps_o.tile([P, 512], F32, tag="ops")
                    po_ps2 = ps_o.tile([P, 512], F32, tag="ops")
                    ot = opool.tile([P, D], F32, tag="ot")
                    for nd, pp in ((0, po_ps), (1, po_ps2)):
                        for kt2 in range(2):
                            nc.tensor.matmul(
                                pp[:],
                                (avT[qg][:, kt2, sti * P : (sti + 1) * P]),
                                (wo_sb[:, kt2, nd * 512 : (nd + 1) * 512]),
                                start=(kt2 == 0),
                                stop=(kt2 == 1),
                            )
                        nc.vector.tensor_copy(
                            ot[:, nd * 512 : (nd + 1) * 512], pp[:]
                        )
                    nc.sync.dma_start(out[st * P : (st + 1) * P, :], ot[:])


_NC_CACHE = None


def get_nc():
    global _NC_CACHE
    if _NC_CACHE is None:
        _NC_CACHE = build_nc()
    return _NC_CACHE


def make_in_maps(x, Wq, bq, Wk, bk, Wv, bv, Wo, bo):
    in_maps = []
    for c in range(8):
        b, g = c // 4, c % 4
        sl = slice(g * DG, (g + 1) * DG)
        in_maps.append(
            {
                "xT": np.ascontiguousarray(x[b].T),
                "wq": np.ascontiguousarray(Wq[:, sl]),
                "wk": np.ascontiguousarray(Wk[:, sl]),
                "wv": np.ascontiguousarray(Wv[:, sl]),
                "wo": np.ascontiguousarray(Wo[sl, :]),
                "bq": np.ascontiguousarray(bq[sl].reshape(2, P).T),
                "bk": np.ascontiguousarray(bk[sl].reshape(2, P).T),
                "bv": np.ascontiguousarray(bv[sl].reshape(1, DG)),
            }
        )
    return in_maps


def kernel(x, Wq, bq, Wk, bk, Wv, bv, Wo, bo, _run_kwargs=None):
    from concourse.bass_utils import run_bass_kernel_spmd

    x = np.asarray(x, dtype=np.float32)
    nc = get_nc()
    in_maps = make_in_maps(
        x,
        np.asarray(Wq, np.float32),
        np.asarray(bq, np.float32),
        np.asarray(Wk, np.float32),
        np.asarray(bk, np.float32),
        np.asarray(Wv, np.float32),
        np.asarray(bv, np.float32),
        np.asarray(Wo, np.float32),
        np.asarray(bo, np.float32),
    )
    res = run_bass_kernel_spmd(
        nc, in_maps, core_ids=list(range(8)), **(_run_kwargs or {})
    )
    bo = np.asarray(bo, np.float32)
    outp = np.empty((2, S, D), dtype=np.float32)
    for b in range(2):
        acc = res.results[4 * b]["out"].astype(np.float32)
        for g in range(1, 4):
            acc = acc + res.results[4 * b + g]["out"]
        outp[b] = acc + bo[None, :]
    kernel.last_result = res
    return outp


# revision 9
# speedup vs baseline: 1.0535x; 1.0535x over previous
"""Entropy-regularized attention (standard MHA fwd) on 8 trn2 cores.

Sharding: core c -> batch b=c//4, head-group g=c%4 (4 of 16 heads).
Each core computes q/k/v for its 256-wide head-group slice, transposed-
layout attention (scores^T = K^T-stationary matmuls, exp on ACT, AV with
v-stationary producing avT), then a row-split Wo partial product.
Host sums the 4 partials per batch and adds bo (the "all-reduce").

All matmuls run as float32r (1 cycle/row vs 4 for plain fp32).
"""

import sys

for _p in ("/opt/trn_rl_repo", "/root/.axon_site/_ro/trn_rl_repo"):
    if _p not in sys.path:
        sys.path.insert(0, _p)

import numpy as np

import concourse.bass as bass
import concourse.mybir as mybir
import concourse.tile as tile
from concourse import bacc

P = 128
S = 2048  # sequence length
D = 1024  # hidden
DG = 256  # per-core head-group width (4 heads x 64)
HD = 64
NHL = 4  # heads per core
KT_D = D // P  # 8 contraction tiles for projections
ST = S // P  # 16 sequence tiles
QG = 1024  # qi group size (PSUM budget: scores 2x2 banks + av 2 + rb 2)
NQG = S // QG

F32 = mybir.dt.float32
F32R = mybir.dt.float32r


def build_nc():
    nc = bacc.Bacc(None, target_bir_lowering=False)

    xT = nc.dram_tensor("xT", [D, S], F32R, kind="ExternalInput")
    wq = nc.dram_tensor("wq", [D, DG], F32R, kind="ExternalInput")
    wk = nc.dram_tensor("wk", [D, DG], F32R, kind="ExternalInput")
    wv = nc.dram_tensor("wv", [D, DG], F32R, kind="ExternalInput")
    wo = nc.dram_tensor("wo", [DG, D], F32R, kind="ExternalInput")
    bq = nc.dram_tensor("bq", [P, 2], F32, kind="ExternalInput")
    bk = nc.dram_tensor("bk", [P, 2], F32, kind="ExternalInput")
    bv = nc.dram_tensor("bv", [1, DG], F32R, kind="ExternalInput")
    out = nc.dram_tensor("out", [S, D], F32, kind="ExternalOutput")

    with tile.TileContext(nc) as tc:
        _body(tc, nc, xT, wq, wk, wv, wo, bq, bk, bv, out)
    nc.compile()
    return nc


def _body(tc, nc, xT, wq, wk, wv, wo, bq, bk, bv, out):
    from contextlib import ExitStack

    with ExitStack() as ctx:
        ctx.enter_context(
            nc.allow_low_precision(
                reason="float32r tiles feed fp32r matmuls; accum stays fp32 in PSUM"
            )
        )
        persist = ctx.enter_context(tc.tile_pool(name="persist", bufs=1))

        qT_sb = persist.tile([P, 2, S], F32R)
        kT_sb = persist.tile([P, 2, S], F32R)
        v_sb = persist.tile([P, ST, NHL * 65], F32R)  # 65-striped: col 64 = ones
        avT = [
            persist.tile([P, 2, QG], F32R, tag=f"avT{g}", name=f"avT{g}")
            for g in range(NQG)
        ]
        wo_sb = persist.tile([P, 2, D], F32R)
        ones_row = persist.tile([1, P], F32R)

        nc.sync.dma_start(wo_sb[:], wo.rearrange("(kt p) n -> p kt n", p=P))
        # memset can't emit float32r; stage fp32 ones and copy-cast (rounds)
        ones_f32 = persist.tile([P, P], F32)
        nc.vector.memset(ones_f32[:], 1.0)
        nc.vector.tensor_copy(ones_row[:], ones_f32[0:1, :])
        nc.vector.tensor_copy(
            v_sb.rearrange("p st (h w) -> p st h w", w=65)[:, :, :, 64],
            ones_f32[:, 0:64].rearrange("p (st h) -> p st h", h=4),
        )

        # ---- Phase B: projections qT/kT [256, S], v [S, 256] ----
        with ExitStack() as bctx:
            wpool = bctx.enter_context(tc.tile_pool(name="wpool", bufs=1))
            xpool = bctx.enter_context(tc.tile_pool(name="xpool", bufs=1))
            psB = bctx.enter_context(
                tc.tile_pool(name="psB", bufs=2, space="PSUM")
            )

            xT_sb = xpool.tile([P, KT_D, S], F32R)
            nc.sync.dma_start(xT_sb[:], xT.rearrange("(kt p) s -> p kt s", p=P))

            wq_sb = wpool.tile([P, KT_D, DG], F32R, tag="wq")
            wk_sb = wpool.tile([P, KT_D, DG], F32R, tag="wk")
            wv_sb = wpool.tile([P, KT_D, DG], F32R, tag="wv")
            nc.sync.dma_start(wq_sb[:], wq.rearrange("(kt p) n -> p kt n", p=P))
            nc.sync.dma_start(wk_sb[:], wk.rearrange("(kt p) n -> p kt n", p=P))
            nc.sync.dma_start(wv_sb[:], wv.rearrange("(kt p) n -> p kt n", p=P))
            bq_sb = wpool.tile([P, 2], F32, tag="bq")
            bk_sb = wpool.tile([P, 2], F32, tag="bk")
            bv_sb = wpool.tile([1, DG], F32R, tag="bv")
            nc.sync.dma_start(bq_sb[:], bq[:])
            nc.sync.dma_start(bk_sb[:], bk[:])
            nc.sync.dma_start(bv_sb[:], bv[:])

            # qT/kT: out[d', s] = W[:, d'].T @ xT ; bias folded into eviction
            for wsb, bsb, dest in ((wq_sb, bq_sb, qT_sb), (wk_sb, bk_sb, kT_sb)):
                for mt in range(2):
                    for nq in range(S // 512):
                        ps = psB.tile([P, 512], F32, tag="qkps")
                        for kt in range(KT_D):
                            nc.tensor.matmul(
                                ps[:],
                                (wsb[:, kt, mt * P : (mt + 1) * P]),
                                (xT_sb[:, kt, nq * 512 : (nq + 1) * 512]),
                                start=(kt == 0),
                                stop=(kt == KT_D - 1),
                            )
                        nc.vector.tensor_scalar_add(
                            dest[:, mt, nq * 512 : (nq + 1) * 512],
                            ps[:],
                            bsb[:, mt : mt + 1],
                        )

            # v natural [s, d'] ; bias via K=1 ones matmul
            for st in range(ST):
                ps = psB.tile([P, DG], F32, tag="vps")
                for kt in range(KT_D):
                    nc.tensor.matmul(
                        ps[:],
                        (xT_sb[:, kt, st * P : (st + 1) * P]),
                        (wv_sb[:, kt, :]),
                        start=(kt == 0),
                        stop=False,
                    )
                nc.tensor.matmul(
                    ps[:],
                    (ones_row[0:1, 0:P]),
                    (bv_sb[0:1, :]),
                    start=False,
                    stop=True,
                )
                nc.vector.tensor_copy(
                    v_sb.rearrange("p st (h w) -> p st h w", w=65)[
                        :, st, :, 0:64
                    ],
                    ps.rearrange("p (h w) -> p h w", w=64),
                )

        # ---- Phase C: attention, + Phase D: projection (overlapped) ----
        with ExitStack() as cctx:
            expool = cctx.enter_context(tc.tile_pool(name="expool", bufs=3))
            npool = cctx.enter_context(tc.tile_pool(name="npool", bufs=2))
            opool = cctx.enter_context(tc.tile_pool(name="opool", bufs=3))
            ps_sc = cctx.enter_context(
                tc.tile_pool(name="ps_sc", bufs=2, space="PSUM")
            )
            ps_av = cctx.enter_context(
                tc.tile_pool(name="ps_av", bufs=1, space="PSUM")
            )
            ps_o = cctx.enter_context(
                tc.tile_pool(name="ps_o", bufs=2, space="PSUM")
            )

            for qg in range(NQG):
                q0 = qg * QG
                for h in range(NHL):
                    mt, po = h // 2, (h % 2) * 64
                    av = ps_av.tile([P, QG], F32, tag="av")
                    for kt in range(ST):
                        sc = ps_sc.tile([P, QG], F32, tag="sc")
                        for nq in range(QG // 512):
                            nc.tensor.matmul(
                                sc[:, nq * 512 : (nq + 1) * 512],
                                (kT_sb[po : po + 64, mt, kt * P : (kt + 1) * P]),
                                (
                                    qT_sb[
                                        po : po + 64,
                                        mt,
                                        q0 + nq * 512 : q0 + (nq + 1) * 512,
                                    ]
                                ),
                                start=True,
                                stop=True,
                            )
                        ex = expool.tile([P, QG], F32R, tag="ex")
                        nc.scalar.activation(
                            ex[:],
                            sc[:],
                            mybir.ActivationFunctionType.Exp,
                            scale=0.125,
                        )
                        for nq in range(QG // 512):
                            nc.tensor.matmul(
                                av[0:65, nq * 512 : (nq + 1) * 512],
                                (v_sb[:, kt, h * 65 : h * 65 + 65]),
                                (ex[:, nq * 512 : (nq + 1) * 512]),
                                start=(kt == 0),
                                stop=(kt == ST - 1),
                            )
                    # normalize: avT_norm = av[0:64] * (1 / av[64]) bcast over d
                    # 1/l via exp(-ln(l)) on ACT: DVE reciprocal is ~6.5us for
                    # [1,1024] (iterative divide) and stalls PE past the HAM
                    # window; ln+exp share one ACT table set and cost ~1.1us ea
                    l_sb = npool.tile([1, QG], F32, tag="lsb")
                    nc.scalar.activation(
                        l_sb[:], av[64:65, :], mybir.ActivationFunctionType.Ln
                    )
                    r_row = npool.tile([1, QG], F32R, tag="rrow")
                    nc.scalar.activation(
                        r_row[:],
                        l_sb[:],
                        mybir.ActivationFunctionType.Exp,
                        scale=-1.0,
                    )
                    rb = ps_sc.tile([P, QG], F32, tag="sc")
                    for nq in range(QG // 512):
                        nc.tensor.matmul(
                            rb[0:64, nq * 512 : (nq + 1) * 512],
                            (ones_row[0:1, 0:64]),
                            (r_row[0:1, nq * 512 : (nq + 1) * 512]),
                            start=True,
                            stop=True,
                        )
                    u_sb = npool.tile([64, QG], F32R, tag="usb")
                    nc.vector.tensor_copy(u_sb[:], av[0:64, :])
                    nc.vector.tensor_mul(
                        out=avT[qg][po : po + 64, mt, :],
                        in0=u_sb[:],
                        in1=rb[0:64, :],
                    )

                # Phase D for this qi-group (st tiles qg*8 .. qg*8+8)
                for sti in range(QG // P):
                    st = qg * (QG // P) + sti
                    po_ps = ps_o.tile([P, 512], F32, tag="ops")
                    po_ps2 = ps_o.tile([P, 512], F32, tag="ops")
                    ot = opool.tile([P, D], F32, tag="ot")
                    for nd, pp in ((0, po_ps), (1, po_ps2)):
                        for kt2 in range(2):
                            nc.tensor.matmul(
                                pp[:],
                                (avT[qg][:, kt2, sti * P : (sti + 1) * P]),
                                (wo_sb[:, kt2, nd * 512 : (nd + 1) * 512]),
                                start=(kt2 == 0),
                                stop=(kt2 == 1),
                            )
                        nc.vector.tensor_copy(
                            ot[:, nd * 512 : (nd + 1) * 512], pp[:]
                        )
                    nc.sync.dma_start(out[st * P : (st + 1) * P, :], ot[:])


_NC_CACHE = None


def get_nc():
    global _NC_CACHE
    if _NC_CACHE is None:
        _NC_CACHE = build_nc()
    return _NC_CACHE


def make_in_maps(x, Wq, bq, Wk, bk, Wv, bv, Wo, bo):
    in_maps = []
    for c in range(8):
        b, g = c // 4, c % 4
        sl = slice(g * DG, (g + 1) * DG)
        in_maps.append(
            {
                "xT": np.ascontiguousarray(x[b].T),
                "wq": np.ascontiguousarray(Wq[:, sl]),
                "wk": np.ascontiguousarray(Wk[:, sl]),
                "wv": np.ascontiguousarray(Wv[:, sl]),
                "wo": np.ascontiguousarray(Wo[sl, :]),
                "bq": np.ascontiguousarray(bq[sl].reshape(2, P).T),
                "bk": np.ascontiguousarray(bk[sl].reshape(2, P).T),
                "bv": np.ascontiguousarray(bv[sl].reshape(1, DG)),
            }
        )
    return in_maps


def kernel(x, Wq, bq, Wk, bk, Wv, bv, Wo, bo, _run_kwargs=None):
    from concourse.bass_utils import run_bass_kernel_spmd

    x = np.asarray(x, dtype=np.float32)
    nc = get_nc()
    in_maps = make_in_maps(
        x,
        np.asarray(Wq, np.float32),
        np.asarray(bq, np.float32),
        np.asarray(Wk, np.float32),
        np.asarray(bk, np.float32),
        np.asarray(Wv, np.float32),
        np.asarray(bv, np.float32),
        np.asarray(Wo, np.float32),
        np.asarray(bo, np.float32),
    )
    res = run_bass_kernel_spmd(
        nc, in_maps, core_ids=list(range(8)), **(_run_kwargs or {})
    )
    bo = np.asarray(bo, np.float32)
    outp = np.empty((2, S, D), dtype=np.float32)
    for b in range(2):
        acc = res.results[4 * b]["out"].astype(np.float32)
        for g in range(1, 4):
            acc = acc + res.results[4 * b + g]["out"]
        outp[b] = acc + bo[None, :]
    kernel.last_result = res
    return outp


# revision 12
# speedup vs baseline: 1.2498x; 1.1863x over previous
"""Entropy-regularized attention (standard MHA fwd) on 8 trn2 cores.

Sharding: core c -> batch b=c//4, head-group g=c%4 (4 of 16 heads).
Each core computes q/k/v for its 256-wide head-group slice, transposed-
layout attention (scores^T = K^T-stationary matmuls, exp on ACT, AV with
v-stationary producing avT), then a row-split Wo partial product.
Host sums the 4 partials per batch and adds bo (the "all-reduce").

All matmuls run as float32r (1 cycle/row vs 4 for plain fp32).
"""

import sys

for _p in ("/opt/trn_rl_repo", "/root/.axon_site/_ro/trn_rl_repo"):
    if _p not in sys.path:
        sys.path.insert(0, _p)

import numpy as np

import concourse.bass as bass
import concourse.mybir as mybir
import concourse.tile as tile
from concourse import bacc

P = 128
S = 2048  # sequence length
D = 1024  # hidden
DG = 256  # per-core head-group width (4 heads x 64)
HD = 64
NHL = 4  # heads per core
KT_D = D // P  # 8 contraction tiles for projections
ST = S // P  # 16 sequence tiles
QG = 1024  # qi group size (PSUM budget: scores 2x2 banks + av 2 + rb 2)
NQG = S // QG

F32 = mybir.dt.float32
F32R = mybir.dt.float32r


def build_nc():
    nc = bacc.Bacc(None, target_bir_lowering=False)

    xT = nc.dram_tensor("xT", [D, S], F32R, kind="ExternalInput")
    wq = nc.dram_tensor("wq", [D, DG], F32R, kind="ExternalInput")
    wk = nc.dram_tensor("wk", [D, DG], F32R, kind="ExternalInput")
    wv = nc.dram_tensor("wv", [D, DG], F32R, kind="ExternalInput")
    wo = nc.dram_tensor("wo", [DG, D], F32R, kind="ExternalInput")
    bq = nc.dram_tensor("bq", [P, 2], F32, kind="ExternalInput")
    bk = nc.dram_tensor("bk", [P, 2], F32, kind="ExternalInput")
    bv = nc.dram_tensor("bv", [1, DG], F32R, kind="ExternalInput")
    out = nc.dram_tensor("out", [S, D], F32, kind="ExternalOutput")

    with tile.TileContext(nc) as tc:
        _body(tc, nc, xT, wq, wk, wv, wo, bq, bk, bv, out)
    nc.compile()
    return nc


def _body(tc, nc, xT, wq, wk, wv, wo, bq, bk, bv, out):
    from contextlib import ExitStack

    with ExitStack() as ctx:
        ctx.enter_context(
            nc.allow_low_precision(
                reason="float32r tiles feed fp32r matmuls; accum stays fp32 in PSUM"
            )
        )
        persist = ctx.enter_context(tc.tile_pool(name="persist", bufs=1))

        qT_sb = persist.tile([P, 2, S], F32R)
        kT_sb = persist.tile([P, 2, S], F32R)
        v_sb = persist.tile([P, ST, NHL * 65], F32R)  # 65-striped: col 64 = ones
        avT = [
            persist.tile([P, 2, QG], F32R, tag=f"avT{g}", name=f"avT{g}")
            for g in range(NQG)
        ]
        wo_sb = persist.tile([P, 2, D], F32R)
        ones_row = persist.tile([1, P], F32R)

        nc.sync.dma_start(wo_sb[:], wo.rearrange("(kt p) n -> p kt n", p=P))

        # Pre-load the ACT table set containing BOTH Exp and Ln so the
        # bacc fixpoint doesn't alternate exp_and_others <-> natural_log
        # (17 x ~1.3us table loads observed otherwise).
        from concourse.hw_specs import get_activation_tables

        table_names = list(get_activation_tables(nc.m.arch).keys())
        set_id = table_names.index("natural_log_exp_and_others")
        with tc.tile_critical():
            nc.scalar.add_instruction(
                mybir.InstLoadActFuncSet(
                    name=nc.get_next_instruction_name(),
                    ins=[],
                    outs=[],
                    act_func_set_id=set_id,
                )
            )

        # memset can't emit float32r; stage fp32 ones and copy-cast (rounds)
        ones_f32 = persist.tile([P, P], F32)
        nc.vector.memset(ones_f32[:], 1.0)
        nc.vector.tensor_copy(ones_row[:], ones_f32[0:1, :])
        nc.vector.tensor_copy(
            v_sb.rearrange("p st (h w) -> p st h w", w=65)[:, :, :, 64],
            ones_f32[:, 0:64].rearrange("p (st h) -> p st h", h=4),
        )

        # ---- Phase B: projections qT/kT [256, S], v [S, 256] ----
        with ExitStack() as bctx:
            wpool = bctx.enter_context(tc.tile_pool(name="wpool", bufs=1))
            xpool = bctx.enter_context(tc.tile_pool(name="xpool", bufs=1))
            psB = bctx.enter_context(
                tc.tile_pool(name="psB", bufs=2, space="PSUM")
            )

            xT_sb = xpool.tile([P, KT_D, S], F32R)
            nc.sync.dma_start(xT_sb[:], xT.rearrange("(kt p) s -> p kt s", p=P))

            wq_sb = wpool.tile([P, KT_D, DG], F32R, tag="wq")
            wk_sb = wpool.tile([P, KT_D, DG], F32R, tag="wk")
            wv_sb = wpool.tile([P, KT_D, DG], F32R, tag="wv")
            nc.sync.dma_start(wq_sb[:], wq.rearrange("(kt p) n -> p kt n", p=P))
            nc.sync.dma_start(wk_sb[:], wk.rearrange("(kt p) n -> p kt n", p=P))
            nc.sync.dma_start(wv_sb[:], wv.rearrange("(kt p) n -> p kt n", p=P))
            bq_sb = wpool.tile([P, 2], F32, tag="bq")
            bk_sb = wpool.tile([P, 2], F32, tag="bk")
            bv_sb = wpool.tile([1, DG], F32R, tag="bv")
            nc.sync.dma_start(bq_sb[:], bq[:])
            nc.sync.dma_start(bk_sb[:], bk[:])
            nc.sync.dma_start(bv_sb[:], bv[:])

            # qT/kT: out[d', s] = W[:, d'].T @ xT ; bias folded into eviction
            for wsb, bsb, dest in ((wq_sb, bq_sb, qT_sb), (wk_sb, bk_sb, kT_sb)):
                for mt in range(2):
                    for nq in range(S // 512):
                        ps = psB.tile([P, 512], F32, tag="qkps")
                        for kt in range(KT_D):
                            nc.tensor.matmul(
                                ps[:],
                                (wsb[:, kt, mt * P : (mt + 1) * P]),
                                (xT_sb[:, kt, nq * 512 : (nq + 1) * 512]),
                                start=(kt == 0),
                                stop=(kt == KT_D - 1),
                            )
                        nc.vector.tensor_scalar_add(
                            dest[:, mt, nq * 512 : (nq + 1) * 512],
                            ps[:],
                            bsb[:, mt : mt + 1],
                        )

            # v natural [s, d'] ; bias via K=1 ones matmul
            for st in range(ST):
                ps = psB.tile([P, DG], F32, tag="vps")
                for kt in range(KT_D):
                    nc.tensor.matmul(
                        ps[:],
                        (xT_sb[:, kt, st * P : (st + 1) * P]),
                        (wv_sb[:, kt, :]),
                        start=(kt == 0),
                        stop=False,
                    )
                nc.tensor.matmul(
                    ps[:],
                    (ones_row[0:1, 0:P]),
                    (bv_sb[0:1, :]),
                    start=False,
                    stop=True,
                )
                nc.vector.tensor_copy(
                    v_sb.rearrange("p st (h w) -> p st h w", w=65)[
                        :, st, :, 0:64
                    ],
                    ps.rearrange("p (h w) -> p h w", w=64),
                )

        # ---- Phase C: attention, + Phase D: projection (overlapped) ----
        with ExitStack() as cctx:
            expool = cctx.enter_context(tc.tile_pool(name="expool", bufs=3))
            npool = cctx.enter_context(tc.tile_pool(name="npool", bufs=2))
            opool = cctx.enter_context(tc.tile_pool(name="opool", bufs=3))
            ps_sc = cctx.enter_context(
                tc.tile_pool(name="ps_sc", bufs=2, space="PSUM")
            )
            ps_av = cctx.enter_context(
                tc.tile_pool(name="ps_av", bufs=1, space="PSUM")
            )
            ps_o = cctx.enter_context(
                tc.tile_pool(name="ps_o", bufs=2, space="PSUM")
            )

            def emit_norm(qg, h, av):
                # normalize: avT_norm = av[0:64] * (1 / av[64]) bcast over d.
                # 1/l via exp(-ln(l)) on ACT: DVE reciprocal is ~6.5us for
                # [1,1024] (iterative divide) and stalls PE past the HAM
                # window; ln+exp share one ACT table set and cost ~1.1us ea.
                mt, po = h // 2, (h % 2) * 64
                l_sb = npool.tile([1, QG], F32, tag="lsb")
                nc.scalar.activation(
                    l_sb[:], av[64:65, :], mybir.ActivationFunctionType.Ln
                )
                r_row = npool.tile([1, QG], F32R, tag="rrow")
                nc.scalar.activation(
                    r_row[:],
                    l_sb[:],
                    mybir.ActivationFunctionType.Exp,
                    scale=-1.0,
                )
                u_sb = npool.tile([64, QG], F32R, tag="usb")
                nc.vector.tensor_copy(u_sb[:], av[0:64, :])
                # rb lives in the ps_o pool (not ps_sc) so the next head's
                # scores aren't starved of sc slots while the mul drains
                for nq in range(QG // 512):
                    rb = ps_o.tile([P, 512], F32, tag="ops", name="rb")
                    nc.tensor.matmul(
                        rb[0:64, :],
                        (ones_row[0:1, 0:64]),
                        (r_row[0:1, nq * 512 : (nq + 1) * 512]),
                        start=True,
                        stop=True,
                    )
                    nc.vector.tensor_mul(
                        out=avT[qg][po : po + 64, mt, nq * 512 : (nq + 1) * 512],
                        in0=u_sb[:, nq * 512 : (nq + 1) * 512],
                        in1=rb[0:64, :],
                    )

            pending_norm = None
            for qg in range(NQG):
                q0 = qg * QG
                for h in range(NHL):
                    mt, po = h // 2, (h % 2) * 64
                    av = ps_av.tile([P, QG], F32, tag="av")
                    for kt in range(ST):
                        sc = ps_sc.tile([P, QG], F32, tag="sc")
                        for nq in range(QG // 512):
                            nc.tensor.matmul(
                                sc[:, nq * 512 : (nq + 1) * 512],
                                (kT_sb[po : po + 64, mt, kt * P : (kt + 1) * P]),
                                (
                                    qT_sb[
                                        po : po + 64,
                                        mt,
                                        q0 + nq * 512 : q0 + (nq + 1) * 512,
                                    ]
                                ),
                                start=True,
                                stop=True,
                            )
                        ex = expool.tile([P, QG], F32R, tag="ex")
                        nc.scalar.activation(
                            ex[:],
                            sc[:],
                            mybir.ActivationFunctionType.Exp,
                            scale=0.125,
                        )
                        # emit the previous head's normalization a couple of
                        # kt into this head: its rb matmul then sits behind
                        # already-runnable scores work in the PE queue instead
                        # of stalling the PE on the ACT ln/exp latency
                        if kt == 2 and pending_norm is not None:
                            pending_norm()
                            pending_norm = None
                        for nq in range(QG // 512):
                            nc.tensor.matmul(
                                av[0:65, nq * 512 : (nq + 1) * 512],
                                (v_sb[:, kt, h * 65 : h * 65 + 65]),
                                (ex[:, nq * 512 : (nq + 1) * 512]),
                                start=(kt == 0),
                                stop=(kt == ST - 1),
                            )
                    pending_norm = (
                        lambda qg=qg, h=h, av=av: emit_norm(qg, h, av)
                    )

                # flush the last head's norm before its qi-group's projection
                if pending_norm is not None:
                    pending_norm()
                    pending_norm = None

                # Phase D for this qi-group (st tiles qg*8 .. qg*8+8)
                for sti in range(QG // P):
                    st = qg * (QG // P) + sti
                    po_ps = ps_o.tile([P, 512], F32, tag="ops")
                    po_ps2 = ps_o.tile([P, 512], F32, tag="ops")
                    ot = opool.tile([P, D], F32, tag="ot")
                    for nd, pp in ((0, po_ps), (1, po_ps2)):
                        for kt2 in range(2):
                            nc.tensor.matmul(
                                pp[:],
                                (avT[qg][:, kt2, sti * P : (sti + 1) * P]),
                                (wo_sb[:, kt2, nd * 512 : (nd + 1) * 512]),
                                start=(kt2 == 0),
                                stop=(kt2 == 1),
                            )
                        nc.vector.tensor_copy(
                            ot[:, nd * 512 : (nd + 1) * 512], pp[:]
                        )
                    nc.sync.dma_start(out[st * P : (st + 1) * P, :], ot[:])


_NC_CACHE = None


def get_nc():
    global _NC_CACHE
    if _NC_CACHE is None:
        _NC_CACHE = build_nc()
    return _NC_CACHE


def make_in_maps(x, Wq, bq, Wk, bk, Wv, bv, Wo, bo):
    in_maps = []
    for c in range(8):
        b, g = c // 4, c % 4
        sl = slice(g * DG, (g + 1) * DG)
        in_maps.append(
            {
                "xT": np.ascontiguousarray(x[b].T),
                "wq": np.ascontiguousarray(Wq[:, sl]),
                "wk": np.ascontiguousarray(Wk[:, sl]),
                "wv": np.ascontiguousarray(Wv[:, sl]),
                "wo": np.ascontiguousarray(Wo[sl, :]),
                "bq": np.ascontiguousarray(bq[sl].reshape(2, P).T),
                "bk": np.ascontiguousarray(bk[sl].reshape(2, P).T),
                "bv": np.ascontiguousarray(bv[sl].reshape(1, DG)),
            }
        )
    return in_maps


def kernel(x, Wq, bq, Wk, bk, Wv, bv, Wo, bo, _run_kwargs=None):
    from concourse.bass_utils import run_bass_kernel_spmd

    x = np.asarray(x, dtype=np.float32)
    nc = get_nc()
    in_maps = make_in_maps(
        x,
        np.asarray(Wq, np.float32),
        np.asarray(bq, np.float32),
        np.asarray(Wk, np.float32),
        np.asarray(bk, np.float32),
        np.asarray(Wv, np.float32),
        np.asarray(bv, np.float32),
        np.asarray(Wo, np.float32),
        np.asarray(bo, np.float32),
    )
    res = run_bass_kernel_spmd(
        nc, in_maps, core_ids=list(range(8)), **(_run_kwargs or {})
    )
    bo = np.asarray(bo, np.float32)
    outp = np.empty((2, S, D), dtype=np.float32)
    for b in range(2):
        acc = res.results[4 * b]["out"].astype(np.float32)
        for g in range(1, 4):
            acc = acc + res.results[4 * b + g]["out"]
        outp[b] = acc + bo[None, :]
    kernel.last_result = res
    return outp
